# revision 11
# baseline (speedup 1.0000x reference)
"""Per-neuron grouped MLP (conv-style) kernel for Trainium2, 8 NeuronCores.

Math (per group d):  h = x[:, d, :] @ W1[d].T + b1[d]; g = gelu(h); out[:, d] = g @ W2[d] + b2[d]
  x: [B=512, D=2048, M=128], W1: [D, H=128, M], b1: [D, H], W2: [D, H], b2: [D]

Strategy (v2):
  - Shard on D: each of 8 cores owns D_LOC = 256 independent per-neuron MLPs.
  - x is quantized to int8 on host with a per-(d,m) scale folded into W1
    (W1'[m,d,h] = W1[d,h,m] * sx[d,m]); the DMA casts int8 -> fp16 in the
    SDMA datapath, so HBM traffic for x is 1 byte/elem while the matmul
    runs in fp16 on integer-valued activations.
  - DRAM layouts are [M, D_LOC, *] so every per-supergroup DMA reads a
    contiguous per-partition chunk (8KB x / 4KB w1).
  - Per pair of 2 d's: 2 matmuls into one [H, 2B] fp32 psum tile, one Gelu
    activation (ScalarE, exact erf) psum -> fp16 g in SBUF.
  - Per quad of 4 d's: 4 packed MM2s, tile_position=(0,32j) -> psum rows
    {0,32,64,96}; DVE copies psum -> fp16 o_sb; strided-partition DMA to
    outT fp16. b2 and the fp32 upcast happen on host.
"""

import numpy as np

B, D, M, H = 512, 2048, 128, 128
N_CORES = 8
D_LOC = D // N_CORES  # 256
QUAD = 4     # d's per MM2 packing group
PAIR = 2     # d's per psum1/ACT batch
SUPER = 16   # d's per super-group: one x DMA, one w1 DMA, one out DMA
# Within a super-group [D0, D0+16), quad c (c=0..3) handles d = D0 + 4j + c
# (j=0..3); MM2 j lands on psum row 32j, so out rows {D0..D0+15} are exactly
# o_sb[0::32, c, :] in (row, quad, b) iteration order -> single strided DMA.

X_INT8 = True  # False: ship x as fp16 (no quantization)

_NC_CACHE = {}


def build_nc(bias_mode: bool, x_int8: bool = X_INT8, reps: int = 1):
    key = (bias_mode, x_int8, reps)
    if key in _NC_CACHE:
        return _NC_CACHE[key]

    import concourse.bacc as bacc
    import concourse.mybir as mybir
    import concourse.tile as tile

    f32 = mybir.dt.float32
    f16 = mybir.dt.float16
    xdt = mybir.dt.int8 if x_int8 else f16
    GELU = mybir.ActivationFunctionType.Gelu

    nc = bacc.Bacc("TRN2", target_bir_lowering=False, debug=False, num_devices=N_CORES)
    xT = nc.dram_tensor("xT", [M, D_LOC, B], xdt, kind="ExternalInput").ap()
    w1T = nc.dram_tensor("w1T", [M, D_LOC, H], f16, kind="ExternalInput").ap()
    w2T = nc.dram_tensor("w2T", [H, D_LOC], f16, kind="ExternalInput").ap()
    b1T = nc.dram_tensor("b1T", [H, D_LOC], f32, kind="ExternalInput").ap()
    outT = nc.dram_tensor("outT", [D_LOC, B], f16, kind="ExternalOutput").ap()

    with (
        tile.TileContext(nc) as tc,
        tc.tile_pool(name="singles", bufs=1) as singles,
        tc.tile_pool(name="xp", bufs=3) as xp,
        tc.tile_pool(name="wp", bufs=2) as wp,
        tc.tile_pool(name="gp", bufs=4) as gp,
        tc.tile_pool(name="op", bufs=4) as op_pool,
        tc.tile_pool(name="ps1", bufs=3, space="PSUM") as ps1,
        tc.tile_pool(name="ps2", bufs=2, space="PSUM") as ps2,
    ):
        w2_sb = singles.tile([H, D_LOC], f16)
        nc.sync.dma_start(out=w2_sb[:], in_=w2T[:])
        b1_sb = None
        if bias_mode:
            b1_sb = singles.tile([H, D_LOC], f32)
            nc.sync.dma_start(out=b1_sb[:], in_=b1T[:])

        for _rep in range(reps):
            _body_loop(nc, tc, bias_mode, f16, f32, GELU,
                       xT, w1T, outT, w2_sb, b1_sb,
                       xp, wp, gp, op_pool, ps1, ps2)

    nc.compile()
    _NC_CACHE[key] = nc
    return nc


def _body_loop(nc, tc, bias_mode, f16, f32, GELU, xT, w1T, outT, w2_sb, b1_sb,
               xp, wp, gp, op_pool, ps1, ps2):
    NQ = SUPER // QUAD   # 4 quads per supergroup
    NSG = D_LOC // SUPER
    NPAIR_Q = QUAD // PAIR  # 2 pairs per quad
    # pair index p -> (sg, c, pr); d = D0 + NQ*(PAIR*pr + j) + c for j in 0..1
    pairs = [
        (sg, c, pr)
        for sg in range(NSG)
        for c in range(NQ)
        for pr in range(NPAIR_Q)
    ]

    sg_state = {}   # sg -> (x_sb, w1_sb, o_sb)
    quad_g = {}     # (sg, c) -> g_sb

    def emit_mm1(pi):
        """Stage 1: (DMA loads at supergroup start) + 2 MM1 matmuls."""
        sg, c, pr = pairs[pi]
        D0 = sg * SUPER
        if c == 0 and pr == 0:
            x_sb = xp.tile([M, SUPER, B], f16)
            nc.gpsimd.dma_start(out=x_sb[:], in_=xT[:, D0 : D0 + SUPER, :])
            w1_sb = wp.tile([M, SUPER, H], f16)
            nc.sync.dma_start(out=w1_sb[:], in_=w1T[:, D0 : D0 + SUPER, :])
            o_sb = op_pool.tile([128, NQ, B], f16)
            sg_state[sg] = (x_sb, w1_sb, o_sb)
        x_sb, w1_sb, _ = sg_state[sg]
        p1 = ps1.tile([H, PAIR * B], f32)
        for j in range(PAIR):
            jj = PAIR * pr + j
            nc.tensor.matmul(
                p1[:, j * B : (j + 1) * B],
                lhsT=w1_sb[:, NQ * jj + c, :],
                rhs=x_sb[:, NQ * jj + c, :],
                start=True,
                stop=True,
            )
        return p1

    def emit_consume(pi, p1):
        """Stage 2: gelu; at quad end also MM2 -> DVE copy -> out DMA."""
        sg, c, pr = pairs[pi]
        D0 = sg * SUPER
        _, _, o_sb = sg_state[sg]
        if pr == 0:
            g_new = gp.tile([H, QUAD * B], f16, name="g_quad")
            quad_g[(sg, c)] = g_new
        g_sb = quad_g[(sg, c)]
        gsl = g_sb[:, pr * PAIR * B : (pr + 1) * PAIR * B]
        if bias_mode:
            for j in range(PAIR):
                dd = D0 + NQ * (PAIR * pr + j) + c
                nc.scalar.activation(
                    gsl[:, j * B : (j + 1) * B],
                    p1[:, j * B : (j + 1) * B],
                    GELU,
                    bias=b1_sb[:, dd : dd + 1],
                )
        else:
            nc.scalar.activation(gsl[:], p1[:], GELU)
        if pr != NPAIR_Q - 1:
            return
        # quad complete: MM2 (4 col-tiled 1-row matmuls) + DVE copy
        del quad_g[(sg, c)]
        p2 = ps2.tile([128, B], f32)
        for j in range(QUAD):
            dd = D0 + NQ * j + c
            nc.tensor.matmul(
                p2[32 * j : 32 * j + 1, :],
                lhsT=w2_sb[:, dd : dd + 1],
                rhs=g_sb[:, j * B : (j + 1) * B],
                start=True,
                stop=True,
                tile_position=(0, 32 * j),
            )
        nc.vector.tensor_copy(o_sb[:, c, :], p2[:])
        if c == NQ - 1:
            nc.sync.dma_start(
                out=outT[D0 : D0 + SUPER, :], in_=o_sb[0::32, :, :]
            )
            del sg_state[sg]

    # 1-deep software pipeline at pair granularity: PE runs MM1(p+1)
    # while ACT consumes pair p.
    prev = emit_mm1(0)
    for pi in range(len(pairs)):
        if pi + 1 < len(pairs):
            nxt = emit_mm1(pi + 1)
        emit_consume(pi, prev)
        prev = nxt if pi + 1 < len(pairs) else None


def prepare_in_maps(x, W1, b1, W2, x_int8: bool = X_INT8):
    """Host-side shard + transpose (+ int8 quantization). 8 per-core dicts."""
    x = np.asarray(x, dtype=np.float32)
    W1 = np.asarray(W1, dtype=np.float32)
    b1 = np.asarray(b1, dtype=np.float32)
    W2 = np.asarray(W2, dtype=np.float32)

    in_maps = []
    for k in range(N_CORES):
        sl = slice(k * D_LOC, (k + 1) * D_LOC)
        xk = x[:, sl, :]  # [B, D_LOC, M]
        w1k = W1[sl]      # [D_LOC, H, M]
        if x_int8:
            sx = np.abs(xk).max(axis=0) / 127.0          # [D_LOC, M]
            sx = np.maximum(sx, 1e-12)
            xq = np.rint(xk / sx[None]).astype(np.int8)  # [B, D_LOC, M]
            xT_k = np.ascontiguousarray(xq.transpose(2, 1, 0))          # [M, D_LOC, B]
            w1s = w1k * sx[:, None, :]                   # [D_LOC, H, M] * sx[d,m]
        else:
            xT_k = np.ascontiguousarray(
                xk.transpose(2, 1, 0), dtype=np.float16
            )
            w1s = w1k
        w1T_k = np.ascontiguousarray(w1s.transpose(2, 0, 1), dtype=np.float16)  # [M, D_LOC, H]
        w2T_k = np.ascontiguousarray(W2[sl].T, dtype=np.float16)
        b1T_k = np.ascontiguousarray(b1[sl].T, dtype=np.float32)
        in_maps.append({"xT": xT_k, "w1T": w1T_k, "w2T": w2T_k, "b1T": b1T_k})
    return in_maps


def assemble_output(results, b2):
    outT_full = np.concatenate([r["outT"] for r in results], axis=0)  # [D, B] f16
    out = outT_full.T.astype(np.float32)  # [B, D]
    b2 = np.asarray(b2, dtype=np.float32)
    if np.any(b2):
        out = out + b2[None, :]
    return np.ascontiguousarray(out)


def kernel(pre_activation_history, W1, b1, W2, b2):
    from concourse.bass_utils import run_bass_kernel_spmd

    b1 = np.asarray(b1, dtype=np.float32)
    bias_mode = bool(np.any(b1))
    nc = build_nc(bias_mode)
    in_maps = prepare_in_maps(pre_activation_history, W1, b1, W2)
    res = run_bass_kernel_spmd(nc, in_maps, core_ids=list(range(N_CORES)))
    return assemble_output(res.results, b2)


# revision 14
# speedup vs baseline: 1.1817x; 1.1817x over previous
"""Per-neuron grouped MLP (conv-style) kernel for Trainium2, 8 NeuronCores.

Math (per group d):  h = x[:, d, :] @ W1[d].T + b1[d]; g = gelu(h); out[:, d] = g @ W2[d] + b2[d]
  x: [B=512, D=2048, M=128], W1: [D, H=128, M], b1: [D, H], W2: [D, H], b2: [D]

Strategy (v2):
  - Shard on D: each of 8 cores owns D_LOC = 256 independent per-neuron MLPs.
  - x is quantized to int8 on host with a per-(d,m) scale folded into W1
    (W1'[m,d,h] = W1[d,h,m] * sx[d,m]); the DMA casts int8 -> fp16 in the
    SDMA datapath, so HBM traffic for x is 1 byte/elem while the matmul
    runs in fp16 on integer-valued activations.
  - DRAM layouts are [M, D_LOC, *] so every per-supergroup DMA reads a
    contiguous per-partition chunk (8KB x / 4KB w1).
  - Per pair of 2 d's: 2 matmuls into one [H, 2B] fp32 psum tile, one Gelu
    activation (ScalarE, exact erf) psum -> fp16 g in SBUF.
  - Per quad of 4 d's: 4 packed MM2s, tile_position=(0,32j) -> psum rows
    {0,32,64,96}; DVE copies psum -> fp16 o_sb; strided-partition DMA to
    outT fp16. b2 and the fp32 upcast happen on host.
"""

import numpy as np

B, D, M, H = 512, 2048, 128, 128
N_CORES = 8
D_LOC = D // N_CORES  # 256
QUAD = 4     # d's per MM2 packing group
PAIR = 2     # d's per psum1/ACT batch
SUPER = 16   # d's per super-group: one x DMA, one w1 DMA, one out DMA
# Within a super-group [D0, D0+16), quad c (c=0..3) handles d = D0 + 4j + c
# (j=0..3); MM2 j lands on psum row 32j, so out rows {D0..D0+15} are exactly
# o_sb[0::32, c, :] in (row, quad, b) iteration order -> single strided DMA.

X_INT8 = True  # False: ship x as fp16 (no quantization)

_NC_CACHE = {}


def build_nc(bias_mode: bool, x_int8: bool = X_INT8, reps: int = 1):
    key = (bias_mode, x_int8, reps)
    if key in _NC_CACHE:
        return _NC_CACHE[key]

    import concourse.bacc as bacc
    import concourse.mybir as mybir
    import concourse.tile as tile

    f32 = mybir.dt.float32
    f16 = mybir.dt.float16
    xdt = mybir.dt.int8 if x_int8 else f16
    GELU = mybir.ActivationFunctionType.Gelu

    nc = bacc.Bacc("TRN2", target_bir_lowering=False, debug=False, num_devices=N_CORES)
    xT = nc.dram_tensor("xT", [M, D_LOC, B], xdt, kind="ExternalInput").ap()
    w1T = nc.dram_tensor("w1T", [M, D_LOC, H], f16, kind="ExternalInput").ap()
    w2T = nc.dram_tensor("w2T", [H, D_LOC], f16, kind="ExternalInput").ap()
    b1T = nc.dram_tensor("b1T", [H, D_LOC], f32, kind="ExternalInput").ap()
    outT = nc.dram_tensor("outT", [D_LOC, B], f16, kind="ExternalOutput").ap()

    with (
        tile.TileContext(nc) as tc,
        tc.tile_pool(name="singles", bufs=1) as singles,
        tc.tile_pool(name="xp", bufs=3) as xp,
        tc.tile_pool(name="wp", bufs=2) as wp,
        tc.tile_pool(name="gp", bufs=4) as gp,
        tc.tile_pool(name="op", bufs=4) as op_pool,
        tc.tile_pool(name="ps1", bufs=3, space="PSUM") as ps1,
        tc.tile_pool(name="ps2", bufs=2, space="PSUM") as ps2,
    ):
        w2_sb = singles.tile([H, D_LOC], f16)
        nc.sync.dma_start(out=w2_sb[:], in_=w2T[:])
        b1_sb = None
        if bias_mode:
            b1_sb = singles.tile([H, D_LOC], f32)
            nc.sync.dma_start(out=b1_sb[:], in_=b1T[:])

        for _rep in range(reps):
            _body_loop(nc, tc, bias_mode, f16, f32, GELU,
                       xT, w1T, outT, w2_sb, b1_sb,
                       xp, wp, gp, op_pool, ps1, ps2)

    nc.compile()
    _NC_CACHE[key] = nc
    return nc


def _sg_plan():
    """Supergroup sizes: small at the start (fast pipeline fill: compute can
    begin after a 262KB DMA instead of 2.1MB) and at the end (short tail)."""
    sizes = [4, 4, 8] + [SUPER] * ((D_LOC - 32) // SUPER) + [8, 4, 4]
    assert sum(sizes) == D_LOC
    out, d0 = [], 0
    for s in sizes:
        out.append((d0, s))
        d0 += s
    return out


def _body_loop(nc, tc, bias_mode, f16, f32, GELU, xT, w1T, outT, w2_sb, b1_sb,
               xp, wp, gp, op_pool, ps1, ps2):
    NPAIR_Q = QUAD // PAIR  # 2 pairs per quad
    sgs = _sg_plan()
    # pair index -> (sg, c, pr); within sg of NQ quads, quad c handles
    # d = D0 + NQ*(PAIR*pr + j) + c
    pairs = [
        (sgi, c, pr)
        for sgi, (D0, size) in enumerate(sgs)
        for c in range(size // QUAD)
        for pr in range(NPAIR_Q)
    ]

    sg_state = {}   # sgi -> (x_sb, w1_sb, o_sb)
    quad_g = {}     # (sgi, c) -> g_sb

    def emit_mm1(pi):
        """Stage 1: (DMA loads at supergroup start) + 2 MM1 matmuls."""
        sgi, c, pr = pairs[pi]
        D0, size = sgs[sgi]
        NQ = size // QUAD
        if c == 0 and pr == 0:
            x_sb = xp.tile([M, size, B], f16, name=f"x_{size}")
            nc.gpsimd.dma_start(out=x_sb[:], in_=xT[:, D0 : D0 + size, :])
            w1_sb = wp.tile([M, size, H], f16, name=f"w1_{size}")
            nc.sync.dma_start(out=w1_sb[:], in_=w1T[:, D0 : D0 + size, :])
            o_sb = op_pool.tile([128, NQ, B], f16, name=f"o_{size}")
            sg_state[sgi] = (x_sb, w1_sb, o_sb)
        x_sb, w1_sb, _ = sg_state[sgi]
        p1 = ps1.tile([H, PAIR * B], f32)
        for j in range(PAIR):
            jj = PAIR * pr + j
            nc.tensor.matmul(
                p1[:, j * B : (j + 1) * B],
                lhsT=w1_sb[:, NQ * jj + c, :],
                rhs=x_sb[:, NQ * jj + c, :],
                start=True,
                stop=True,
            )
        return p1

    def emit_consume(pi, p1):
        """Stage 2: gelu; at quad end also MM2 -> DVE copy -> out DMA."""
        sgi, c, pr = pairs[pi]
        D0, size = sgs[sgi]
        NQ = size // QUAD
        _, _, o_sb = sg_state[sgi]
        if pr == 0:
            g_new = gp.tile([H, QUAD * B], f16, name="g_quad")
            quad_g[(sgi, c)] = g_new
        g_sb = quad_g[(sgi, c)]
        gsl = g_sb[:, pr * PAIR * B : (pr + 1) * PAIR * B]
        if bias_mode:
            for j in range(PAIR):
                dd = D0 + NQ * (PAIR * pr + j) + c
                nc.scalar.activation(
                    gsl[:, j * B : (j + 1) * B],
                    p1[:, j * B : (j + 1) * B],
                    GELU,
                    bias=b1_sb[:, dd : dd + 1],
                )
        else:
            nc.scalar.activation(gsl[:], p1[:], GELU)
        if pr != NPAIR_Q - 1:
            return
        # quad complete: MM2 (4 col-tiled 1-row matmuls) + DVE copy
        del quad_g[(sgi, c)]
        p2 = ps2.tile([128, B], f32)
        for j in range(QUAD):
            dd = D0 + NQ * j + c
            nc.tensor.matmul(
                p2[32 * j : 32 * j + 1, :],
                lhsT=w2_sb[:, dd : dd + 1],
                rhs=g_sb[:, j * B : (j + 1) * B],
                start=True,
                stop=True,
                tile_position=(0, 32 * j),
            )
        nc.vector.tensor_copy(o_sb[:, c, :], p2[:])
        if c == NQ - 1:
            nc.sync.dma_start(
                out=outT[D0 : D0 + size, :], in_=o_sb[0::32, :, :]
            )
            del sg_state[sgi]

    # 1-deep software pipeline at pair granularity: PE runs MM1(p+1)
    # while ACT consumes pair p.
    prev = emit_mm1(0)
    for pi in range(len(pairs)):
        if pi + 1 < len(pairs):
            nxt = emit_mm1(pi + 1)
        emit_consume(pi, prev)
        prev = nxt if pi + 1 < len(pairs) else None


def prepare_in_maps(x, W1, b1, W2, x_int8: bool = X_INT8):
    """Host-side shard + transpose (+ int8 quantization). 8 per-core dicts."""
    x = np.asarray(x, dtype=np.float32)
    W1 = np.asarray(W1, dtype=np.float32)
    b1 = np.asarray(b1, dtype=np.float32)
    W2 = np.asarray(W2, dtype=np.float32)

    in_maps = []
    for k in range(N_CORES):
        sl = slice(k * D_LOC, (k + 1) * D_LOC)
        xk = x[:, sl, :]  # [B, D_LOC, M]
        w1k = W1[sl]      # [D_LOC, H, M]
        if x_int8:
            sx = np.abs(xk).max(axis=0) / 127.0          # [D_LOC, M]
            sx = np.maximum(sx, 1e-12)
            xq = np.rint(xk / sx[None]).astype(np.int8)  # [B, D_LOC, M]
            xT_k = np.ascontiguousarray(xq.transpose(2, 1, 0))          # [M, D_LOC, B]
            w1s = w1k * sx[:, None, :]                   # [D_LOC, H, M] * sx[d,m]
        else:
            xT_k = np.ascontiguousarray(
                xk.transpose(2, 1, 0), dtype=np.float16
            )
            w1s = w1k
        w1T_k = np.ascontiguousarray(w1s.transpose(2, 0, 1), dtype=np.float16)  # [M, D_LOC, H]
        w2T_k = np.ascontiguousarray(W2[sl].T, dtype=np.float16)
        b1T_k = np.ascontiguousarray(b1[sl].T, dtype=np.float32)
        in_maps.append({"xT": xT_k, "w1T": w1T_k, "w2T": w2T_k, "b1T": b1T_k})
    return in_maps


def assemble_output(results, b2):
    outT_full = np.concatenate([r["outT"] for r in results], axis=0)  # [D, B] f16
    out = outT_full.T.astype(np.float32)  # [B, D]
    b2 = np.asarray(b2, dtype=np.float32)
    if np.any(b2):
        out = out + b2[None, :]
    return np.ascontiguousarray(out)


def kernel(pre_activation_history, W1, b1, W2, b2):
    from concourse.bass_utils import run_bass_kernel_spmd

    b1 = np.asarray(b1, dtype=np.float32)
    bias_mode = bool(np.any(b1))
    nc = build_nc(bias_mode)
    in_maps = prepare_in_maps(pre_activation_history, W1, b1, W2)
    res = run_bass_kernel_spmd(nc, in_maps, core_ids=list(range(N_CORES)))
    return assemble_output(res.results, b2)
